# revision 12
# baseline (speedup 1.0000x reference)
"""AlignConLoss on 8 TRN2 NeuronCores.

loss = sum_j [ logsumexp_i sim[i,j] ] - sum_j sim[j,j]
with sim = l2norm(enc2) @ l2norm(enc1).T   (B=8192, D=256, T=1)

Distribution: the BxB similarity matrix is sharded row-wise (contrast rows,
enc2) across the 8 cores.  Every core receives the full anchor matrix (enc1)
in its own HBM, so no anchor all-gather is needed.  Each core computes its
1024xB block of sim with j (anchors) on PSUM partitions and i (contrast) on
the free axis, applies exp via the ScalarE activation (folding the anchor
1/||a_j|| into the activation's per-partition scale) with a fused per-column
accumulation, and the per-column partial sums + diagonal partials are
combined across cores with a single small AllGather.

Dataflow per core:
  enc1 f32 --gpsimd cast DMA--> bf16 DRAM --xbar DMA transpose--> aT sbuf
  enc2 shard f32 --cast DMA--> sbuf, row-normalized (norms via fused
    multiply+reduce), bounced through DRAM, xbar-transposed --> cnT sbuf
  anchor row norms: each core computes its own shard's norms and they are
    shared via a tiny AllGather (4KB), 1/sqrt computed as exp(-0.5*ln(x)) so
    only one ACT table set is ever loaded.
  64 j-tiles: 4 bf16 matmuls [128x128] @ [128x512] -> psum [128,1024],
    one Exp activation with accum_out -> column partials.
  Final: AllGather of [128, 65] partials, local sum/log/subtract, and a
    [128,1] x [128,1] matmul reduces partitions to the scalar loss.
"""

import numpy as np

import concourse.bass as bass
import concourse.mybir as mybir
import concourse.tile as tile
from concourse import bacc
from concourse.bass_utils import run_bass_kernel_spmd

P = 128          # partitions
B = 8192         # batch (anchors = contrast = B)
D = 256          # embedding dim
M = 8            # cores
SH = B // M      # 1024 rows per shard
ST = SH // P     # 8 row-tiles per shard
NT = B // P      # 64 j-tiles
DH = D // P      # 2 contraction chunks of 128
IC = 512         # moving-operand free-dim chunk

F32 = mybir.dt.float32
BF16 = mybir.dt.bfloat16
AF = mybir.ActivationFunctionType
ALU = mybir.AluOpType
AX = mybir.AxisListType

REPLICAS = [list(range(M))]


def build_kernel(stage: str = "full") -> bacc.Bacc:
    nc = bacc.Bacc(
        "TRN2",
        target_bir_lowering=False,
        debug=False,
        num_devices=M,
    )
    a_ext = nc.dram_tensor("a", [B, D], F32, kind="ExternalInput").ap()
    c_ext = nc.dram_tensor("c", [SH, D], F32, kind="ExternalInput").ap()
    s_ext = nc.dram_tensor("a_s", [SH, D], F32, kind="ExternalInput").ap()
    out_ext = nc.dram_tensor("out", [1, 1], F32, kind="ExternalOutput").ap()

    with tile.TileContext(nc) as tc:
        _body(tc, nc, a_ext, c_ext, s_ext, out_ext, stage)

    nc.compile()
    return nc


def _body(tc, nc, a_ext, c_ext, s_ext, out_ext, stage="full"):
    with (
        tc.tile_pool(name="const", bufs=1) as const,
        tc.tile_pool(name="work", bufs=2) as work,
        tc.tile_pool(name="scr", bufs=3) as scr,
        tc.tile_pool(name="mm_psum", bufs=3, space="PSUM") as mm_psum,
        tc.tile_pool(name="fin_psum", bufs=1, space="PSUM") as fin_psum,
        tc.tile_pool(name="dram", bufs=1, space="DRAM") as dram,
    ):
        # ---- persistent SBUF tensors
        # aT[s][p, h, n] = bf16(a[s*SH + n, h*P + p]); one slab per shard so
        # matmuls only wait on the slab they read.
        aT = [
            const.tile([P, DH, SH], BF16, tag=f"aT{s}", name=f"aT{s}")
            for s in range(M)
        ]
        cnT = const.tile([P, DH, SH], BF16, tag="cnT")
        c_nat = const.tile([P, ST, D], BF16, tag="c_nat")
        cn_nat = const.tile([P, ST, D], BF16, tag="cn_nat")
        s_nat = const.tile([P, ST, D], BF16, tag="s_nat")
        cnorm2 = const.tile([P, ST], F32, tag="cnorm2")
        snorm2 = const.tile([P, ST], F32, tag="snorm2")
        lnc = const.tile([P, ST], F32, tag="lnc")
        lns = const.tile([P, ST], F32, tag="lns")
        rinv_c = const.tile([P, ST], F32, tag="rinv_c")
        rinv_s = const.tile([P, ST], F32, tag="rinv_s")
        anorm2 = const.tile([P, NT], F32, tag="anorm2")
        lna = const.tile([P, NT], F32, tag="lna")
        rinva = const.tile([P, NT], F32, tag="rinva")
        colpart = const.tile([P, NT], F32, tag="colpart")
        diagp = const.tile([P, ST], F32, tag="diagp")
        diagacc = const.tile([P, 1], F32, tag="diagacc")
        ones = const.tile([P, 1], F32, tag="ones")

        # ---- DRAM bounce buffers
        a_bf = [
            dram.tile([SH, D], BF16, tag=f"a_bf{s}", name=f"a_bf{s}")
            for s in range(M)
        ]
        cn_dram = dram.tile([SH, D], BF16, tag="cn_dram")
        agn_in = dram.tile([P, ST], F32, tag="agn_in")
        agn_out = dram.tile([M * P, ST], F32, tag="agn_out")
        agp_in = dram.tile([P, NT + 1], F32, tag="agp_in")
        agp_out = dram.tile([M * P, NT + 1], F32, tag="agp_out")

        nc.vector.memset(ones[:], 1.0)

        # ---- anchor pipeline: f32 -> bf16 (cast DMA) -> xbar transpose
        for s in range(M):
            nc.gpsimd.dma_start(
                out=a_bf[s][:], in_=a_ext[s * SH : (s + 1) * SH, :]
            )
            for h in range(DH):
                nc.sync.dma_start_transpose(
                    aT[s][:, h, :], a_bf[s][:, h * P : (h + 1) * P]
                )

        # ---- contrast shard + anchor shard natural layouts (cast DMAs)
        nc.gpsimd.dma_start(
            out=c_nat[:], in_=c_ext.rearrange("(t p) d -> p t d", p=P)
        )
        nc.gpsimd.dma_start(
            out=s_nat[:], in_=s_ext.rearrange("(t p) d -> p t d", p=P)
        )

        # ---- row norms (fused square+rowsum), per row-tile
        for t in range(ST):
            sq = scr.tile([P, D], BF16, tag="sq")
            nc.vector.scalar_tensor_tensor(
                out=sq[:],
                in0=c_nat[:, t],
                scalar=1.0,
                in1=c_nat[:, t],
                op0=ALU.mult,
                op1=ALU.mult,
                accum_out=cnorm2[:, t : t + 1],
            )
            sq2 = scr.tile([P, D], BF16, tag="sq")
            nc.vector.scalar_tensor_tensor(
                out=sq2[:],
                in0=s_nat[:, t],
                scalar=1.0,
                in1=s_nat[:, t],
                op0=ALU.mult,
                op1=ALU.mult,
                accum_out=snorm2[:, t : t + 1],
            )

        if stage == "prep_cs":
            chk = work.tile([P, 1], F32, tag="chk")
            nc.vector.reduce_sum(out=chk[:], in_=cnorm2[:], axis=AX.X)
            nc.vector.tensor_add(out=chk[:], in0=chk[:], in1=snorm2[:, 0:1])
            for s in range(M):
                nc.vector.tensor_add(
                    out=chk[:], in0=chk[:], in1=aT[s][:, 0, 0:1]
                )
            nc.sync.dma_start(out=out_ext, in_=chk[0:1, 0:1])
            return

        # ---- share anchor shard norms: AllGather [128, 8] -> [1024, 8]
        if stage != "prep_noag":
            nc.sync.dma_start(out=agn_in[:], in_=snorm2[:])
            nc.gpsimd.collective_compute(
                "AllGather",
                ALU.bypass,
                replica_groups=REPLICAS,
                ins=[agn_in[:].opt()],
                outs=[agn_out[:].opt()],
            )
            # anorm2[p, m*ST + t] = agn_out[m*P + p, t]
            nc.sync.dma_start(
                out=anorm2[:].rearrange("p (m t) -> p m t", m=M),
                in_=agn_out[:].rearrange("(m p) t -> p m t", p=P),
            )
        else:
            # stand-in: tile this core's own shard norms across all 64 slots
            for mm in range(M):
                nc.vector.tensor_copy(
                    out=anorm2[:, mm * ST : (mm + 1) * ST], in_=snorm2[:]
                )

        # ---- 1/sqrt via exp(-0.5 * ln x): stays in one ACT table set
        nc.scalar.activation(out=lnc[:], in_=cnorm2[:], func=AF.Ln)
        nc.scalar.activation(out=rinv_c[:], in_=lnc[:], func=AF.Exp, scale=-0.5)
        nc.scalar.activation(out=lns[:], in_=snorm2[:], func=AF.Ln)
        nc.scalar.activation(out=rinv_s[:], in_=lns[:], func=AF.Exp, scale=-0.5)
        nc.scalar.activation(out=lna[:], in_=anorm2[:], func=AF.Ln)
        nc.scalar.activation(out=rinva[:], in_=lna[:], func=AF.Exp, scale=-0.5)

        # ---- normalize contrast rows, bounce through DRAM, transpose
        for t in range(ST):
            nc.vector.tensor_scalar_mul(
                out=cn_nat[:, t], in0=c_nat[:, t], scalar1=rinv_c[:, t : t + 1]
            )
        nc.sync.dma_start(
            out=cn_dram[:].rearrange("(t p) d -> p t d", p=P), in_=cn_nat[:]
        )
        for h in range(DH):
            nc.sync.dma_start_transpose(
                cnT[:, h, :], cn_dram[:, h * P : (h + 1) * P]
            )

        # ---- diagonal partials: sim[j,j] for this shard's j
        # diagp[p,t] = (1/||a_j||) * sum_d cn[j,d] * a_raw[j,d]
        for t in range(ST):
            sq3 = scr.tile([P, D], BF16, tag="sq")
            nc.vector.scalar_tensor_tensor(
                out=sq3[:],
                in0=cn_nat[:, t],
                scalar=rinv_s[:, t : t + 1],
                in1=s_nat[:, t],
                op0=ALU.mult,
                op1=ALU.mult,
                accum_out=diagp[:, t : t + 1],
            )
        nc.vector.reduce_sum(out=diagacc[:], in_=diagp[:], axis=AX.X)

        if stage in ("prep", "prep_noag"):
            # drain: touch every prep result so nothing is dead-coded
            chk = work.tile([P, 1], F32, tag="chk")
            nc.vector.reduce_sum(out=chk[:], in_=rinva[:], axis=AX.X)
            nc.vector.tensor_add(out=chk[:], in0=chk[:], in1=diagacc[:])
            for s in range(M):
                nc.vector.tensor_add(
                    out=chk[:], in0=chk[:], in1=aT[s][:, 0, 0:1]
                )
            nc.vector.tensor_add(out=chk[:], in0=chk[:], in1=cnT[:, 0, 0:1])
            nc.sync.dma_start(out=out_ext, in_=chk[0:1, 0:1])
            return

        # ---- main loop: 64 j-tiles
        for jt in range(NT):
            s, jloc = jt // ST, (jt % ST) * P
            ps = mm_psum.tile([P, 2 * IC], F32, tag="mmps")
            for h in range(DH):
                w = aT[s][:, h, jloc : jloc + P]
                first, last = h == 0, h == DH - 1
                nc.tensor.matmul(
                    ps[:, 0:IC], w, cnT[:, h, 0:IC], start=first, stop=last
                )
                nc.tensor.matmul(
                    ps[:, IC : 2 * IC],
                    w,
                    cnT[:, h, IC : 2 * IC],
                    start=first,
                    stop=last,
                )
            ex = scr.tile([P, 2 * IC], BF16, tag="expout")
            nc.scalar.activation(
                out=ex[:],
                in_=ps[:],
                func=AF.Exp,
                scale=rinva[:, jt : jt + 1],
                accum_out=colpart[:, jt : jt + 1],
            )

        if stage == "nofinal":
            chk = work.tile([P, 1], F32, tag="chk")
            nc.vector.reduce_sum(out=chk[:], in_=colpart[:], axis=AX.X)
            nc.vector.tensor_add(out=chk[:], in0=chk[:], in1=diagacc[:])
            nc.sync.dma_start(out=out_ext, in_=chk[0:1, 0:1])
            return

        # ---- cross-core combine: AllGather [128, 65] -> [1024, 65]
        nc.sync.dma_start(out=agp_in[:, 0:NT], in_=colpart[:])
        nc.sync.dma_start(out=agp_in[:, NT : NT + 1], in_=diagacc[:])
        nc.gpsimd.collective_compute(
            "AllGather",
            ALU.bypass,
            replica_groups=REPLICAS,
            ins=[agp_in[:].opt()],
            outs=[agp_out[:].opt()],
        )
        gath = work.tile([P, M, NT + 1], F32, tag="gath")
        nc.sync.dma_start(
            out=gath[:], in_=agp_out[:].rearrange("(m p) f -> p m f", p=P)
        )
        S = work.tile([P, NT + 1], F32, tag="Ssum")
        nc.vector.reduce_sum(
            out=S[:], in_=gath[:].rearrange("p m f -> p f m"), axis=AX.X
        )
        lg = work.tile([P, NT], F32, tag="lg")
        lsum = work.tile([P, 1], F32, tag="lsum")
        nc.scalar.activation(
            out=lg[:], in_=S[:, 0:NT], func=AF.Ln, accum_out=lsum[:]
        )
        val = work.tile([P, 1], F32, tag="val")
        nc.vector.tensor_sub(out=val[:], in0=lsum[:], in1=S[:, NT : NT + 1])

        # ---- partition reduction to a scalar: ones.T-weighted matmul
        pres = fin_psum.tile([1, 1], F32, tag="pres")
        nc.tensor.matmul(pres[:], val[:], ones[:], start=True, stop=True)
        outsb = work.tile([1, 1], F32, tag="outsb")
        nc.vector.tensor_copy(out=outsb[:], in_=pres[:])
        nc.sync.dma_start(out=out_ext, in_=outsb[:])


_NC_CACHE = None


def _get_nc():
    global _NC_CACHE
    if _NC_CACHE is None:
        _NC_CACHE = build_kernel()
    return _NC_CACHE


def kernel(**inputs) -> np.ndarray:
    a = np.ascontiguousarray(
        np.asarray(inputs["encoder_embedding1"], dtype=np.float32)
    )
    c = np.ascontiguousarray(
        np.asarray(inputs["encoder_embedding2"], dtype=np.float32)
    )
    assert a.shape == (B, D) and c.shape == (B, D)

    nc = _get_nc()
    in_maps = [
        {
            "a": a,
            "c": c[m * SH : (m + 1) * SH],
            "a_s": a[m * SH : (m + 1) * SH],
        }
        for m in range(M)
    ]
    res = run_bass_kernel_spmd(nc, in_maps, core_ids=list(range(M)))
    return np.float32(res.results[0]["out"][0, 0])


# revision 16
# speedup vs baseline: 1.0902x; 1.0902x over previous
"""AlignConLoss on 8 TRN2 NeuronCores.

loss = sum_j [ logsumexp_i sim[i,j] ] - sum_j sim[j,j]
with sim = l2norm(enc2) @ l2norm(enc1).T   (B=8192, D=256, T=1)

Distribution: the BxB similarity matrix is sharded row-wise (contrast rows,
enc2) across the 8 cores.  Every core receives the full anchor matrix (enc1)
in its own HBM, so no anchor all-gather is needed.  Each core computes its
1024xB block of sim with j (anchors) on PSUM partitions and i (contrast) on
the free axis, applies exp via the ScalarE activation (folding the anchor
1/||a_j|| into the activation's per-partition scale) with a fused per-column
accumulation, and the per-column partial sums + diagonal partials are
combined across cores with a single small AllGather.

Dataflow per core:
  enc1 f32 --gpsimd cast DMA--> bf16 DRAM --xbar DMA transpose--> aT sbuf
  enc2 shard f32 --cast DMA--> sbuf, row-normalized (norms via fused
    multiply+reduce), bounced through DRAM, xbar-transposed --> cnT sbuf
  anchor row norms: each core computes its own shard's norms and they are
    shared via a tiny AllGather (4KB), 1/sqrt computed as exp(-0.5*ln(x)) so
    only one ACT table set is ever loaded.
  64 j-tiles: 4 bf16 matmuls [128x128] @ [128x512] -> psum [128,1024],
    one Exp activation with accum_out -> column partials.
  Final: AllGather of [128, 65] partials, local sum/log/subtract, and a
    [128,1] x [128,1] matmul reduces partitions to the scalar loss.
"""

import numpy as np

import concourse.bass as bass
import concourse.mybir as mybir
import concourse.tile as tile
from concourse import bacc
from concourse.bass_utils import run_bass_kernel_spmd

P = 128          # partitions
B = 8192         # batch (anchors = contrast = B)
D = 256          # embedding dim
M = 8            # cores
SH = B // M      # 1024 rows per shard
ST = SH // P     # 8 row-tiles per shard
NT = B // P      # 64 j-tiles
DH = D // P      # 2 contraction chunks of 128
IC = 512         # moving-operand free-dim chunk

F32 = mybir.dt.float32
BF16 = mybir.dt.bfloat16
AF = mybir.ActivationFunctionType
ALU = mybir.AluOpType
AX = mybir.AxisListType

REPLICAS = [list(range(M))]


def build_kernel(stage: str = "full") -> bacc.Bacc:
    nc = bacc.Bacc(
        "TRN2",
        target_bir_lowering=False,
        debug=False,
        num_devices=M,
    )
    a_ext = nc.dram_tensor("a", [B, D], F32, kind="ExternalInput").ap()
    c_ext = nc.dram_tensor("c", [SH, D], F32, kind="ExternalInput").ap()
    s_ext = nc.dram_tensor("a_s", [SH, D], F32, kind="ExternalInput").ap()
    out_ext = nc.dram_tensor("out", [1, 1], F32, kind="ExternalOutput").ap()

    with tile.TileContext(nc) as tc:
        _body(tc, nc, a_ext, c_ext, s_ext, out_ext, stage)

    nc.compile()
    return nc


def _body(tc, nc, a_ext, c_ext, s_ext, out_ext, stage="full"):
    with (
        tc.tile_pool(name="const", bufs=1) as const,
        tc.tile_pool(name="work", bufs=2) as work,
        tc.tile_pool(name="scr", bufs=3) as scr,
        tc.tile_pool(name="mm_psum", bufs=3, space="PSUM") as mm_psum,
        tc.tile_pool(name="fin_psum", bufs=1, space="PSUM") as fin_psum,
        tc.tile_pool(name="dram", bufs=1, space="DRAM") as dram,
    ):
        # ---- persistent SBUF tensors
        # aT[s][p, h, n] = bf16(a[s*SH + n, h*P + p]); one slab per shard so
        # matmuls only wait on the slab they read.
        aT = [
            const.tile([P, DH, SH], BF16, tag=f"aT{s}", name=f"aT{s}")
            for s in range(M)
        ]
        cnT = const.tile([P, DH, SH], BF16, tag="cnT")
        c_nat = const.tile([P, ST, D], BF16, tag="c_nat")
        cn_nat = const.tile([P, ST, D], BF16, tag="cn_nat")
        s_nat = const.tile([P, ST, D], BF16, tag="s_nat")
        cnorm2 = const.tile([P, ST], F32, tag="cnorm2")
        snorm2 = const.tile([P, ST], F32, tag="snorm2")
        lnc = const.tile([P, ST], F32, tag="lnc")
        lns = const.tile([P, ST], F32, tag="lns")
        rinv_c = const.tile([P, ST], F32, tag="rinv_c")
        rinv_s = const.tile([P, ST], F32, tag="rinv_s")
        anorm2 = const.tile([P, NT], F32, tag="anorm2")
        lna = const.tile([P, NT], F32, tag="lna")
        rinva = const.tile([P, NT], F32, tag="rinva")
        colpart = const.tile([P, NT], F32, tag="colpart")
        diagp = const.tile([P, ST], F32, tag="diagp")
        diagacc = const.tile([P, 1], F32, tag="diagacc")
        ones = const.tile([P, 1], F32, tag="ones")

        # ---- DRAM bounce buffers
        a_bf = [
            dram.tile([SH, D], BF16, tag=f"a_bf{s}", name=f"a_bf{s}")
            for s in range(M)
        ]
        cn_dram = dram.tile([SH, D], BF16, tag="cn_dram")
        agn_in = dram.tile([P, ST], F32, tag="agn_in")
        agn_out = dram.tile([M * P, ST], F32, tag="agn_out")
        agp_in = dram.tile([P, NT + 1], F32, tag="agp_in")
        agp_out = dram.tile([M * P, NT + 1], F32, tag="agp_out")

        nc.vector.memset(ones[:], 1.0)

        # ---- contrast shard + anchor shard natural layouts (cast DMAs).
        # These come FIRST: the norms AllGather and the cnT chain gate the
        # main loop, while the bulk anchor casts can trail behind.
        nc.gpsimd.dma_start(
            out=c_nat[:], in_=c_ext.rearrange("(t p) d -> p t d", p=P)
        )
        nc.gpsimd.dma_start(
            out=s_nat[:], in_=s_ext.rearrange("(t p) d -> p t d", p=P)
        )

        # ---- row norms (fused square+rowsum), per row-tile
        for t in range(ST):
            sq = scr.tile([P, D], BF16, tag="sq")
            nc.vector.scalar_tensor_tensor(
                out=sq[:],
                in0=c_nat[:, t],
                scalar=1.0,
                in1=c_nat[:, t],
                op0=ALU.mult,
                op1=ALU.mult,
                accum_out=cnorm2[:, t : t + 1],
            )
            sq2 = scr.tile([P, D], BF16, tag="sq")
            nc.vector.scalar_tensor_tensor(
                out=sq2[:],
                in0=s_nat[:, t],
                scalar=1.0,
                in1=s_nat[:, t],
                op0=ALU.mult,
                op1=ALU.mult,
                accum_out=snorm2[:, t : t + 1],
            )

        if stage == "prep_cs":
            chk = work.tile([P, 1], F32, tag="chk")
            nc.vector.reduce_sum(out=chk[:], in_=cnorm2[:], axis=AX.X)
            nc.vector.tensor_add(out=chk[:], in0=chk[:], in1=snorm2[:, 0:1])
            for s in range(M):
                nc.vector.tensor_add(
                    out=chk[:], in0=chk[:], in1=aT[s][:, 0, 0:1]
                )
            nc.sync.dma_start(out=out_ext, in_=chk[0:1, 0:1])
            return

        # ---- share anchor shard norms: AllGather [128, 8] -> [1024, 8]
        # The readback DMA waits on the collective; issue it from the scalar
        # HWDGE queue so the wait doesn't stall the sync queue's transposes.
        if stage != "prep_noag":
            nc.sync.dma_start(out=agn_in[:], in_=snorm2[:])
            nc.gpsimd.collective_compute(
                "AllGather",
                ALU.bypass,
                replica_groups=REPLICAS,
                ins=[agn_in[:].opt()],
                outs=[agn_out[:].opt()],
            )
            # anorm2[p, m*ST + t] = agn_out[m*P + p, t]
            nc.scalar.dma_start(
                out=anorm2[:].rearrange("p (m t) -> p m t", m=M),
                in_=agn_out[:].rearrange("(m p) t -> p m t", p=P),
            )
        else:
            # stand-in: tile this core's own shard norms across all 64 slots
            for mm in range(M):
                nc.vector.tensor_copy(
                    out=anorm2[:, mm * ST : (mm + 1) * ST], in_=snorm2[:]
                )

        # ---- 1/sqrt via exp(-0.5 * ln x): stays in one ACT table set
        nc.scalar.activation(out=lnc[:], in_=cnorm2[:], func=AF.Ln)
        nc.scalar.activation(out=rinv_c[:], in_=lnc[:], func=AF.Exp, scale=-0.5)
        nc.scalar.activation(out=lns[:], in_=snorm2[:], func=AF.Ln)
        nc.scalar.activation(out=rinv_s[:], in_=lns[:], func=AF.Exp, scale=-0.5)
        nc.scalar.activation(out=lna[:], in_=anorm2[:], func=AF.Ln)
        nc.scalar.activation(out=rinva[:], in_=lna[:], func=AF.Exp, scale=-0.5)

        # ---- normalize contrast rows, bounce through DRAM, transpose
        for t in range(ST):
            nc.vector.tensor_scalar_mul(
                out=cn_nat[:, t], in0=c_nat[:, t], scalar1=rinv_c[:, t : t + 1]
            )
        nc.sync.dma_start(
            out=cn_dram[:].rearrange("(t p) d -> p t d", p=P), in_=cn_nat[:]
        )
        for h in range(DH):
            nc.sync.dma_start_transpose(
                cnT[:, h, :], cn_dram[:, h * P : (h + 1) * P]
            )

        # ---- diagonal partials: sim[j,j] for this shard's j
        # diagp[p,t] = (1/||a_j||) * sum_d cn[j,d] * a_raw[j,d]
        for t in range(ST):
            sq3 = scr.tile([P, D], BF16, tag="sq")
            nc.vector.scalar_tensor_tensor(
                out=sq3[:],
                in0=cn_nat[:, t],
                scalar=rinv_s[:, t : t + 1],
                in1=s_nat[:, t],
                op0=ALU.mult,
                op1=ALU.mult,
                accum_out=diagp[:, t : t + 1],
            )
        nc.vector.reduce_sum(out=diagacc[:], in_=diagp[:], axis=AX.X)

        # ---- anchor pipeline: f32 -> bf16 (cast DMA) -> xbar transpose
        for s in range(M):
            nc.gpsimd.dma_start(
                out=a_bf[s][:], in_=a_ext[s * SH : (s + 1) * SH, :]
            )
            for h in range(DH):
                nc.sync.dma_start_transpose(
                    aT[s][:, h, :], a_bf[s][:, h * P : (h + 1) * P]
                )

        if stage in ("prep", "prep_noag"):
            # drain: touch every prep result so nothing is dead-coded
            chk = work.tile([P, 1], F32, tag="chk")
            nc.vector.reduce_sum(out=chk[:], in_=rinva[:], axis=AX.X)
            nc.vector.tensor_add(out=chk[:], in0=chk[:], in1=diagacc[:])
            for s in range(M):
                nc.vector.tensor_add(
                    out=chk[:], in0=chk[:], in1=aT[s][:, 0, 0:1]
                )
            nc.vector.tensor_add(out=chk[:], in0=chk[:], in1=cnT[:, 0, 0:1])
            nc.sync.dma_start(out=out_ext, in_=chk[0:1, 0:1])
            return

        # ---- main loop: 64 j-tiles
        for jt in range(NT):
            s, jloc = jt // ST, (jt % ST) * P
            ps = mm_psum.tile([P, 2 * IC], F32, tag="mmps")
            for h in range(DH):
                w = aT[s][:, h, jloc : jloc + P]
                first, last = h == 0, h == DH - 1
                nc.tensor.matmul(
                    ps[:, 0:IC], w, cnT[:, h, 0:IC], start=first, stop=last
                )
                nc.tensor.matmul(
                    ps[:, IC : 2 * IC],
                    w,
                    cnT[:, h, IC : 2 * IC],
                    start=first,
                    stop=last,
                )
            # exp in place on PSUM: only the accum (column sums) is consumed
            nc.scalar.activation(
                out=ps[:],
                in_=ps[:],
                func=AF.Exp,
                scale=rinva[:, jt : jt + 1],
                accum_out=colpart[:, jt : jt + 1],
            )

        if stage == "nofinal":
            chk = work.tile([P, 1], F32, tag="chk")
            nc.vector.reduce_sum(out=chk[:], in_=colpart[:], axis=AX.X)
            nc.vector.tensor_add(out=chk[:], in0=chk[:], in1=diagacc[:])
            nc.sync.dma_start(out=out_ext, in_=chk[0:1, 0:1])
            return

        # ---- cross-core combine: AllGather [128, 65] -> [1024, 65]
        nc.sync.dma_start(out=agp_in[:, 0:NT], in_=colpart[:])
        nc.sync.dma_start(out=agp_in[:, NT : NT + 1], in_=diagacc[:])
        nc.gpsimd.collective_compute(
            "AllGather",
            ALU.bypass,
            replica_groups=REPLICAS,
            ins=[agp_in[:].opt()],
            outs=[agp_out[:].opt()],
        )
        gath = work.tile([P, M, NT + 1], F32, tag="gath")
        nc.sync.dma_start(
            out=gath[:], in_=agp_out[:].rearrange("(m p) f -> p m f", p=P)
        )
        S = work.tile([P, NT + 1], F32, tag="Ssum")
        nc.vector.reduce_sum(
            out=S[:], in_=gath[:].rearrange("p m f -> p f m"), axis=AX.X
        )
        lg = work.tile([P, NT], F32, tag="lg")
        lsum = work.tile([P, 1], F32, tag="lsum")
        nc.scalar.activation(
            out=lg[:], in_=S[:, 0:NT], func=AF.Ln, accum_out=lsum[:]
        )
        val = work.tile([P, 1], F32, tag="val")
        nc.vector.tensor_sub(out=val[:], in0=lsum[:], in1=S[:, NT : NT + 1])

        # ---- partition reduction to a scalar: ones.T-weighted matmul
        pres = fin_psum.tile([1, 1], F32, tag="pres")
        nc.tensor.matmul(pres[:], val[:], ones[:], start=True, stop=True)
        outsb = work.tile([1, 1], F32, tag="outsb")
        nc.vector.tensor_copy(out=outsb[:], in_=pres[:])
        nc.sync.dma_start(out=out_ext, in_=outsb[:])


_NC_CACHE = None


def _get_nc():
    global _NC_CACHE
    if _NC_CACHE is None:
        _NC_CACHE = build_kernel()
    return _NC_CACHE


def kernel(**inputs) -> np.ndarray:
    a = np.ascontiguousarray(
        np.asarray(inputs["encoder_embedding1"], dtype=np.float32)
    )
    c = np.ascontiguousarray(
        np.asarray(inputs["encoder_embedding2"], dtype=np.float32)
    )
    assert a.shape == (B, D) and c.shape == (B, D)

    nc = _get_nc()
    in_maps = [
        {
            "a": a,
            "c": c[m * SH : (m + 1) * SH],
            "a_s": a[m * SH : (m + 1) * SH],
        }
        for m in range(M)
    ]
    res = run_bass_kernel_spmd(nc, in_maps, core_ids=list(range(M)))
    return np.float32(res.results[0]["out"][0, 0])
